# revision 1
# baseline (speedup 1.0000x reference)
"""Self-contained 8-core Trainium2 Bass kernel for nn_MultiHeadAttention.

Problem: x:[4,2048,1024] f32, w_qkv:[3072,1024], b_qkv:[3072],
w_proj:[1024,1024], b_proj:[1024].  16 heads, head_dim 64.

Sharding: core c = batch(4) x head-group(2).  Each core computes QKV for
its 8 heads on its batch, attention, and a partial output projection over
its 512 head-dims.  Host sums the two partials per batch and adds b_proj.

Per-core dataflow (all matmuls bf16, fp32 PSUM):
  - host supplies xT_aug [1152,2048] = [x_b^T; ones; 0pad] (contraction dim
    carries the bias via an augmented row), wqk [1152,1024] with column
    layout head h -> cols h*128..h*128+64 = q (pre-scaled 1/8), +64..+128 = k,
    wv [1152,512], wp [512,1024].
  - qkT[h] tiles [128,2048]: partitions 0:64 = q^T, 64:128 = k^T (d on
    partitions) -> energy^T = k @ q^T computed directly in [k,q] layout, so
    softmax exp output is already the att^T operand for att@V (no PE
    transposes anywhere).
  - v kept natural [n, 65] per head with a ones column: att@V with M=65
    yields out^T rows 0:64 and the softmax denominator in row 64 for free.
  - energies are bounded (~|2|) so exp needs no max subtraction.
  - denominators: psum row 64 -> DRAM bounce -> packed [32,512] -> one
    lane-parallel DVE reciprocal -> stream_shuffle broadcast -> one
    scalar_tensor_tensor multiply normalizes out^T.
"""
import sys

sys.path.insert(0, "/opt/trn_rl_repo")

import numpy as np
import ml_dtypes

import concourse.bass as bass
import concourse.mybir as mybir
import concourse.tile as tile
from concourse import bacc
from concourse.bass_utils import run_bass_kernel_spmd

bf16 = ml_dtypes.bfloat16
F32 = mybir.dt.float32
BF16 = mybir.dt.bfloat16

B, N, EMB = 4, 2048, 1024
HEADS, HD = 16, 64
HPC = 8            # heads per core
KAUG = 1152        # 1024 emb + 1 bias row, padded to 9*128
NKT = KAUG // 128  # 9 contraction tiles for qkv
NT_N = N // 128    # 16 n-tiles
EXPF = mybir.ActivationFunctionType.Exp


def _build_kernel(ctx, tc, nc, xT, wqk, wv, wp, y):
    mult = mybir.AluOpType.mult

    const = ctx.enter_context(tc.tile_pool(name="const", bufs=1))
    qkp = ctx.enter_context(tc.tile_pool(name="qkp", bufs=1))
    vp = ctx.enter_context(tc.tile_pool(name="vp", bufs=1))
    outp = ctx.enter_context(tc.tile_pool(name="outp", bufs=1))
    attp = ctx.enter_context(tc.tile_pool(name="attp", bufs=4))
    misc = ctx.enter_context(tc.tile_pool(name="misc", bufs=1))
    stage = ctx.enter_context(tc.tile_pool(name="stage", bufs=2))
    bcp = ctx.enter_context(tc.tile_pool(name="bcp", bufs=3))
    yp = ctx.enter_context(tc.tile_pool(name="yp", bufs=4))
    pe = ctx.enter_context(tc.tile_pool(name="pe", bufs=3, space="PSUM"))
    po = ctx.enter_context(tc.tile_pool(name="po", bufs=2, space="PSUM"))
    dramp = ctx.enter_context(tc.tile_pool(name="dramp", bufs=1, space="DRAM"))

    # ---- load inputs: xT+wv first (v-phase deps), wqk next, wp last;
    # spread across three DMA queues so the prologue isn't one-queue bound ----
    xT_t = []
    wqk_t = []
    wv_t = []
    for kt in range(NKT):
        t = const.tile([128, N], BF16, tag=f"xT{kt}", name=f"xT{kt}")
        (nc.sync if kt % 2 == 0 else nc.scalar).dma_start(
            t[:], xT[kt * 128:(kt + 1) * 128, :])
        xT_t.append(t)
        t = const.tile([128, 512], BF16, tag=f"wv{kt}", name=f"wv{kt}")
        nc.gpsimd.dma_start(t[:], wv[kt * 128:(kt + 1) * 128, :])
        wv_t.append(t)
    for kt in range(NKT):
        t = const.tile([128, 1024], BF16, tag=f"wqk{kt}", name=f"wqk{kt}")
        nc.gpsimd.dma_start(t[:], wqk[kt * 128:(kt + 1) * 128, :])
        wqk_t.append(t)
    wp_t = []
    for t4 in range(4):
        t = const.tile([128, 1024], BF16, tag=f"wp{t4}", name=f"wp{t4}")
        nc.gpsimd.dma_start(t[:], wp[t4 * 128:(t4 + 1) * 128, :])
        wp_t.append(t)

    # qk bias vectors: aug row 1024 of wqk, one [128,1] per m-tile
    bq_t = []
    for t8 in range(8):
        tb = misc.tile([128, 1], BF16, tag=f"bqb{t8}", name=f"bqb{t8}")
        nc.gpsimd.dma_start(tb[:], wqk[EMB:EMB + 1, t8 * 128:(t8 + 1) * 128])
        t = misc.tile([128, 1], F32, tag=f"bq{t8}", name=f"bq{t8}")
        nc.vector.tensor_copy(t[:], tb[:])
        bq_t.append(t)

    # qk m-tiles 0..3 = q of head pairs (h%2 on partition halves), 4..7 = k.
    qkT = [qkp.tile([128, N], BF16, tag=f"qkT{h}", name=f"qkT{h}") for h in range(HPC)]
    v_t = [vp.tile([128, HPC, 65], BF16, tag=f"v{nt}", name=f"v{nt}") for nt in range(NT_N)]
    outT_raw = [outp.tile([128, N], BF16, tag=f"or{t}", name=f"or{t}") for t in range(4)]
    outT_n = [outp.tile([128, N], BF16, tag=f"on{t}", name=f"on{t}") for t in range(4)]
    den_dram = dramp.tile([32, 512], F32, name="den_dram")
    den_pk = misc.tile([32, 512], F32, tag="den", name="den_pk")
    rec_pk = misc.tile([32, 512], F32, tag="rec", name="rec_pk")
    nc.vector.memset(den_pk[:], 1.0)

    def emit_v_tile(nt):
        p = pe.tile([128, 1024], F32, tag="pe", name="pep")
        for kt in range(NKT):
            nc.tensor.matmul(
                p[:, 0:512],
                xT_t[kt][:, nt * 128:(nt + 1) * 128],
                wv_t[kt][:],
                start=(kt == 0), stop=(kt == NKT - 1),
            )
        nc.scalar.copy(
            v_t[nt][:, :, 0:64],
            p[:, 0:512].rearrange("p (h c) -> p h c", c=64),
        )
        nc.vector.memset(v_t[nt][:, :, 64:65], 1.0)

    def qk_pair_gen(hp):
        """Generator emitting one PE matmul per step for q/k m-tiles of pair hp."""
        for t in (hp, 4 + hp):
            for nbp in range(2):
                p = pe.tile([128, 1024], F32, tag="pe", name="pep")
                for j in range(2):
                    for kt in range(NKT - 1):
                        nc.tensor.matmul(
                            p[:, j * 512:(j + 1) * 512],
                            wqk_t[kt][:, t * 128:(t + 1) * 128],
                            xT_t[kt][:, nbp * 1024 + j * 512:
                                     nbp * 1024 + (j + 1) * 512],
                            start=(kt == 0), stop=(kt == NKT - 2),
                        )
                        if not (j == 1 and kt == NKT - 2):
                            yield
                nc.vector.tensor_scalar_add(
                    qkT[t][:, nbp * 1024:(nbp + 1) * 1024], p[:], bq_t[t][:])
                yield

    def norm_step(h, qg):
        pb = (h % 2) * 64
        bc = bcp.tile([128, 512], F32, tag="bc", name="bc")
        m = [h * 4 + qg] * 32
        nc.vector.stream_shuffle(bc[pb:pb + 32, :], rec_pk[0:32, :], mask=m)
        nc.vector.stream_shuffle(bc[pb + 32:pb + 64, :], rec_pk[0:32, :], mask=m)
        nc.vector.scalar_tensor_tensor(
            outT_n[h // 2][pb:pb + 64, qg * 512:(qg + 1) * 512],
            outT_raw[h // 2][pb:pb + 64, qg * 512:(qg + 1) * 512],
            1.0,
            bc[pb:pb + 64, :],
            op0=mult, op1=mult,
        )

    def normalize_gen(heads):
        for qg in range(4):
            for h in heads:
                norm_step(h, qg)
                yield

    def den_recip(nrows):
        nc.gpsimd.dma_start(den_pk[0:nrows, :], den_dram[0:nrows, :])
        nc.vector.reciprocal(rec_pk[:], den_pk[:])

    # ---- prologue: v tiles, then qk for head-pair 0 (PE-only, ACT idle) ----
    for nt in range(NT_N):
        emit_v_tile(nt)
    for _ in qk_pair_gen(0):
        pass

    # ---- attention, software-pipelined: attv lags one kt behind energy/exp
    # so exp latency is hidden; qk matmuls for the NEXT head pair are
    # interleaved as fillers into the leftover PE slack. ----
    filler = iter(())
    dve_filler = iter(())
    for h in range(HPC):
        if h % 2 == 0 and h // 2 + 1 < 4:
            for _ in filler:  # drain any leftover before switching
                pass
            filler = qk_pair_gen(h // 2 + 1)
        pb0 = (h % 2) * 64
        for qh in range(2):
            o0 = po.tile([128, 512], F32, tag="po", name="o0")
            o1 = po.tile([128, 512], F32, tag="po", name="o1")
            prev_at = None
            for kt in range(NT_N):
                p = pe.tile([128, 1024], F32, tag="pe", name="pep")
                for j in range(2):
                    nc.tensor.matmul(
                        p[:, j * 512:(j + 1) * 512],
                        qkT[4 + h // 2][pb0:pb0 + 64, kt * 128:(kt + 1) * 128],
                        qkT[h // 2][pb0:pb0 + 64,
                                    qh * 1024 + j * 512: qh * 1024 + (j + 1) * 512],
                        start=True, stop=True,
                    )
                at = attp.tile([128, 1024], BF16, tag="att", name="at")
                nc.scalar.activation(at[:], p[:], EXPF)
                if prev_at is not None:
                    for j, o in enumerate((o0, o1)):
                        nc.tensor.matmul(
                            o[0:65, :],
                            v_t[kt - 1][:, h, :],
                            prev_at[:, j * 512:(j + 1) * 512],
                            start=(kt - 1 == 0), stop=False,
                        )
                prev_at = at
                next(filler, None)
                if kt % 4 == 0:
                    next(dve_filler, None)
            for j, o in enumerate((o0, o1)):
                nc.tensor.matmul(
                    o[0:65, :],
                    v_t[NT_N - 1][:, h, :],
                    prev_at[:, j * 512:(j + 1) * 512],
                    start=False, stop=True,
                )
            for j, o in enumerate((o0, o1)):
                qg = qh * 2 + j
                r = h * 4 + qg
                # denominator row -> DRAM bounce
                st = stage.tile([128, 512], F32, tag="st", name="st")
                nc.vector.tensor_copy(st[64:65, :], o[64:65, :])
                nc.gpsimd.dma_start(den_dram[r:r + 1, :], st[64:65, :])
                # raw out^T -> sbuf bf16 (packed 2 heads / tile)
                nc.vector.tensor_copy(
                    outT_raw[h // 2][pb0:pb0 + 64, qg * 512:(qg + 1) * 512],
                    o[0:64, :],
                )
            if h == 7:
                # tail head: den reload + reciprocal + normalize per q-half so
                # proj can start on the lower q range while the upper half of
                # attention drains.  (reload/recip are idempotent recomputes.)
                den_recip(30 if qh == 0 else 32)
                for qg in (2 * qh, 2 * qh + 1):
                    norm_step(7, qg)
        if h % 2 == 1 and h < 7:
            # pair p complete: reload den rows 0..(p+1)*8 (old rows identical)
            # and recompute the full reciprocal tile -- idempotent, keeps all
            # partition starts 32-aligned.
            p_ = h // 2
            den_recip((p_ + 1) * 8)
            for _ in dve_filler:  # drain previous normalize batch
                pass
            dve_filler = normalize_gen((2 * p_, 2 * p_ + 1))
        elif h == 6:
            den_recip(28)
            for _ in dve_filler:
                pass
            dve_filler = normalize_gen((6,))
    for _ in filler:
        pass
    for _ in dve_filler:
        pass

    # ---- phase 3: partial proj  y = outT_n^T @ wp ----
    for nt in range(NT_N):
        ys = yp.tile([128, 1024], F32, tag="y", name="ys")
        for ng in range(2):
            p = po.tile([128, 512], F32, tag="po", name="pp")
            for t4 in range(4):
                nc.tensor.matmul(
                    p[:],
                    outT_n[t4][:, nt * 128:(nt + 1) * 128],
                    wp_t[t4][:, ng * 512:(ng + 1) * 512],
                    start=(t4 == 0), stop=(t4 == 3),
                )
            if ng == 0:
                nc.vector.tensor_copy(ys[:, 0:512], p[:])
            else:
                nc.scalar.copy(ys[:, 512:1024], p[:])
        nc.sync.dma_start(y[nt * 128:(nt + 1) * 128, :], ys[:])


_CACHE = {}


def _get_nc():
    if "nc" not in _CACHE:
        nc = bacc.Bacc("TRN2", target_bir_lowering=False, debug=False, num_devices=8)
        xT = nc.dram_tensor("xT", [KAUG, N], BF16, kind="ExternalInput")
        wqk = nc.dram_tensor("wqk", [KAUG, 1024], BF16, kind="ExternalInput")
        wv = nc.dram_tensor("wv", [KAUG, 512], BF16, kind="ExternalInput")
        wp = nc.dram_tensor("wp", [512, 1024], BF16, kind="ExternalInput")
        y = nc.dram_tensor("y", [N, EMB], F32, kind="ExternalOutput")
        with tile.TileContext(nc) as tc:
            from contextlib import ExitStack
            with ExitStack() as es:
                _build_kernel(es, tc, nc, xT.ap(), wqk.ap(), wv.ap(), wp.ap(), y.ap())
        nc.compile()
        _CACHE["nc"] = nc
    return _CACHE["nc"]


def make_in_maps(x, w_qkv, b_qkv, w_proj):
    """Host-side shard prep: per-core bf16 operands with folded biases/scale."""
    x = np.asarray(x, np.float32)
    w_qkv = np.asarray(w_qkv, np.float32)
    b_qkv = np.asarray(b_qkv, np.float32)
    w_proj = np.asarray(w_proj, np.float32)
    scale = 1.0 / np.sqrt(HD)

    in_maps = []
    for c in range(8):
        b, g = divmod(c, 2)
        heads = range(g * HPC, (g + 1) * HPC)

        xT_aug = np.zeros((KAUG, N), np.float32)
        xT_aug[0:EMB, :] = x[b].T
        xT_aug[EMB, :] = 1.0

        wqk = np.zeros((KAUG, 1024), np.float32)
        wv = np.zeros((KAUG, 512), np.float32)
        for hl, H in enumerate(heads):
            qs, ks, vs = H * HD, EMB + H * HD, 2 * EMB + H * HD
            # q cols: m-tile hl//2, partition half hl%2; k cols: m-tile 4+hl//2
            qc = (hl // 2) * 128 + (hl % 2) * 64
            kc = 512 + qc
            wqk[0:EMB, qc:qc + 64] = w_qkv[qs:qs + HD, :].T * scale
            wqk[EMB, qc:qc + 64] = b_qkv[qs:qs + HD] * scale
            wqk[0:EMB, kc:kc + 64] = w_qkv[ks:ks + HD, :].T
            wqk[EMB, kc:kc + 64] = b_qkv[ks:ks + HD]
            wv[0:EMB, hl * 64:(hl + 1) * 64] = w_qkv[vs:vs + HD, :].T
            wv[EMB, hl * 64:(hl + 1) * 64] = b_qkv[vs:vs + HD]

        wp = w_proj[:, g * 512:(g + 1) * 512].T.copy()

        in_maps.append({
            "xT": xT_aug.astype(bf16),
            "wqk": wqk.astype(bf16),
            "wv": wv.astype(bf16),
            "wp": wp.astype(bf16),
        })
    return in_maps


def kernel(x, w_qkv, b_qkv, w_proj, b_proj):
    x = np.asarray(x, np.float32)
    b_proj = np.asarray(b_proj, np.float32)
    nc = _get_nc()
    in_maps = make_in_maps(x, w_qkv, b_qkv, w_proj)
    res = run_bass_kernel_spmd(nc, in_maps, core_ids=list(range(8)))
    out = np.empty((B, N, EMB), np.float32)
    for b in range(B):
        out[b] = res.results[2 * b]["y"] + res.results[2 * b + 1]["y"] + b_proj
    return out



# revision 7
# speedup vs baseline: 283.8457x; 283.8457x over previous
"""Self-contained 8-core Trainium2 Bass kernel for nn_MultiHeadAttention.

Problem: x:[4,2048,1024] f32, w_qkv:[3072,1024], b_qkv:[3072],
w_proj:[1024,1024], b_proj:[1024].  16 heads, head_dim 64.

Sharding: core c = batch(4) x head-group(2).  Each core computes QKV for
its 8 heads on its batch, attention, and a partial output projection over
its 512 head-dims.  Host sums the two partials per batch and adds b_proj.

Per-core dataflow (all matmuls bf16, fp32 PSUM):
  - host supplies xT_aug [1152,2048] = [x_b^T; ones; 0pad] (contraction dim
    carries the bias via an augmented row), wqk [1152,1024] with column
    layout head h -> cols h*128..h*128+64 = q (pre-scaled 1/8), +64..+128 = k,
    wv [1152,512], wp [512,1024].
  - qkT[h] tiles [128,2048]: partitions 0:64 = q^T, 64:128 = k^T (d on
    partitions) -> energy^T = k @ q^T computed directly in [k,q] layout, so
    softmax exp output is already the att^T operand for att@V (no PE
    transposes anywhere).
  - v kept natural [n, 65] per head with a ones column: att@V with M=65
    yields out^T rows 0:64 and the softmax denominator in row 64 for free.
  - energies are bounded (~|2|) so exp needs no max subtraction.
  - denominators: psum row 64 -> DRAM bounce -> packed [32,512] -> one
    lane-parallel DVE reciprocal -> stream_shuffle broadcast -> one
    scalar_tensor_tensor multiply normalizes out^T.
"""
import sys

sys.path.insert(0, "/opt/trn_rl_repo")

import numpy as np
import ml_dtypes

import concourse.bass as bass
import concourse.mybir as mybir
import concourse.tile as tile
from concourse import bacc
from concourse.bass_utils import run_bass_kernel_spmd

bf16 = ml_dtypes.bfloat16
F32 = mybir.dt.float32
BF16 = mybir.dt.bfloat16

B, N, EMB = 4, 2048, 1024
HEADS, HD = 16, 64
HPC = 8            # heads per core
KAUG = 1152        # 1024 emb + 1 bias row, padded to 9*128
NKT = KAUG // 128  # 9 contraction tiles for qkv
NT_N = N // 128    # 16 n-tiles
EXPF = mybir.ActivationFunctionType.Exp


def _build_kernel(ctx, tc, nc, xT, wqk, wv, wp, y, repeat=1):
    mult = mybir.AluOpType.mult

    const = ctx.enter_context(tc.tile_pool(name="const", bufs=1))
    qkp = ctx.enter_context(tc.tile_pool(name="qkp", bufs=1))
    vp = ctx.enter_context(tc.tile_pool(name="vp", bufs=1))
    outp = ctx.enter_context(tc.tile_pool(name="outp", bufs=1))
    attp = ctx.enter_context(tc.tile_pool(name="attp", bufs=4))
    misc = ctx.enter_context(tc.tile_pool(name="misc", bufs=1))
    stage = ctx.enter_context(tc.tile_pool(name="stage", bufs=2))
    bcp = ctx.enter_context(tc.tile_pool(name="bcp", bufs=3))
    yp = ctx.enter_context(tc.tile_pool(name="yp", bufs=4))
    pe = ctx.enter_context(tc.tile_pool(name="pe", bufs=3, space="PSUM"))
    po = ctx.enter_context(tc.tile_pool(name="po", bufs=2, space="PSUM"))
    dramp = ctx.enter_context(tc.tile_pool(name="dramp", bufs=1, space="DRAM"))

    # qk m-tiles 0..3 = q of head pairs (h%2 on partition halves), 4..7 = k.
    qkT = [qkp.tile([128, N], BF16, tag=f"qkT{h}", name=f"qkT{h}") for h in range(HPC)]
    v_t = [vp.tile([128, HPC, 65], BF16, tag=f"v{nt}", name=f"v{nt}") for nt in range(NT_N)]
    outT_raw = [outp.tile([128, N], BF16, tag=f"or{t}", name=f"or{t}") for t in range(4)]
    outT_n = [outp.tile([128, N], BF16, tag=f"on{t}", name=f"on{t}") for t in range(4)]
    den_dram = dramp.tile([32, 512], F32, name="den_dram")
    den_pk = misc.tile([32, 512], F32, tag="den", name="den_pk")
    rec_pk = misc.tile([32, 512], F32, tag="rec", name="rec_pk")

    def load_inputs():
        # ---- load inputs: xT+wv first (v-phase deps), wqk next, wp last;
        # spread across three DMA queues so the prologue isn't one-queue bound
        xT_t = []
        wqk_t = []
        wv_t = []
        for kt in range(NKT):
            t = const.tile([128, N], BF16, tag=f"xT{kt}", name=f"xT{kt}")
            (nc.sync if kt % 2 == 0 else nc.scalar).dma_start(
                t[:], xT[kt * 128:(kt + 1) * 128, :])
            xT_t.append(t)
            t = const.tile([128, 512], BF16, tag=f"wv{kt}", name=f"wv{kt}")
            nc.gpsimd.dma_start(t[:], wv[kt * 128:(kt + 1) * 128, :])
            wv_t.append(t)
        for kt in range(NKT):
            t = const.tile([128, 1024], BF16, tag=f"wqk{kt}", name=f"wqk{kt}")
            nc.gpsimd.dma_start(t[:], wqk[kt * 128:(kt + 1) * 128, :])
            wqk_t.append(t)
        wp_t = []
        for t4 in range(4):
            t = const.tile([128, 1024], BF16, tag=f"wp{t4}", name=f"wp{t4}")
            nc.gpsimd.dma_start(t[:], wp[t4 * 128:(t4 + 1) * 128, :])
            wp_t.append(t)

        # qk bias vectors: aug row 1024 of wqk, one [128,1] per m-tile
        bq_t = []
        for t8 in range(8):
            tb = misc.tile([128, 1], BF16, tag=f"bqb{t8}", name=f"bqb{t8}")
            nc.gpsimd.dma_start(tb[:], wqk[EMB:EMB + 1, t8 * 128:(t8 + 1) * 128])
            t = misc.tile([128, 1], F32, tag=f"bq{t8}", name=f"bq{t8}")
            nc.vector.tensor_copy(t[:], tb[:])
            bq_t.append(t)
        return xT_t, wqk_t, wv_t, wp_t, bq_t

    xT_t, wqk_t, wv_t, wp_t, bq_t = load_inputs()
    nc.vector.memset(den_pk[:], 1.0)

    def emit_v_tile(nt):
        p = pe.tile([128, 1024], F32, tag="pe", name="pep")
        for kt in range(NKT):
            nc.tensor.matmul(
                p[:, 0:512],
                xT_t[kt][:, nt * 128:(nt + 1) * 128],
                wv_t[kt][:],
                start=(kt == 0), stop=(kt == NKT - 1),
            )
        nc.scalar.copy(
            v_t[nt][:, :, 0:64],
            p[:, 0:512].rearrange("p (h c) -> p h c", c=64),
        )
        nc.vector.memset(v_t[nt][:, :, 64:65], 1.0)

    def qk_pair_gen(hp):
        """Generator emitting one PE matmul per step for q/k m-tiles of pair hp."""
        for t in (hp, 4 + hp):
            for nbp in range(2):
                p = pe.tile([128, 1024], F32, tag="pe", name="pep")
                for j in range(2):
                    for kt in range(NKT - 1):
                        nc.tensor.matmul(
                            p[:, j * 512:(j + 1) * 512],
                            wqk_t[kt][:, t * 128:(t + 1) * 128],
                            xT_t[kt][:, nbp * 1024 + j * 512:
                                     nbp * 1024 + (j + 1) * 512],
                            start=(kt == 0), stop=(kt == NKT - 2),
                        )
                        if not (j == 1 and kt == NKT - 2):
                            yield
                nc.vector.tensor_scalar_add(
                    qkT[t][:, nbp * 1024:(nbp + 1) * 1024], p[:], bq_t[t][:])
                yield

    def norm_step(h, qg):
        pb = (h % 2) * 64
        bc = bcp.tile([128, 512], F32, tag="bc", name="bc")
        m = [h * 4 + qg] * 32
        nc.vector.stream_shuffle(bc[pb:pb + 32, :], rec_pk[0:32, :], mask=m)
        nc.vector.stream_shuffle(bc[pb + 32:pb + 64, :], rec_pk[0:32, :], mask=m)
        nc.vector.scalar_tensor_tensor(
            outT_n[h // 2][pb:pb + 64, qg * 512:(qg + 1) * 512],
            outT_raw[h // 2][pb:pb + 64, qg * 512:(qg + 1) * 512],
            1.0,
            bc[pb:pb + 64, :],
            op0=mult, op1=mult,
        )

    def normalize_gen(heads):
        for qg in range(4):
            for h in heads:
                norm_step(h, qg)
                yield

    def den_recip(nrows):
        nc.gpsimd.dma_start(den_pk[0:nrows, :], den_dram[0:nrows, :])
        nc.vector.reciprocal(rec_pk[:], den_pk[:])

    for _rep in range(repeat):
      if _rep > 0:
        xT_t, wqk_t, wv_t, wp_t, bq_t = load_inputs()
      # ---- prologue: v tiles, then qk for head-pair 0 (PE-only, ACT idle) ----
      for nt in range(NT_N):
        emit_v_tile(nt)
      for _ in qk_pair_gen(0):
        pass

      # ---- attention, software-pipelined: attv lags one kt behind energy/exp
      # so exp latency is hidden; qk matmuls for the NEXT head pair are
      # interleaved as fillers into the leftover PE slack. ----
      filler = iter(())
      dve_filler = iter(())
      for h in range(HPC):
        if h % 2 == 0 and h // 2 + 1 < 4:
            for _ in filler:  # drain any leftover before switching
                pass
            filler = qk_pair_gen(h // 2 + 1)
        pb0 = (h % 2) * 64
        for qh in range(2):
            o0 = po.tile([128, 512], F32, tag="po", name="o0")
            o1 = po.tile([128, 512], F32, tag="po", name="o1")
            prev_at = None
            for kt in range(NT_N):
                p = pe.tile([128, 1024], F32, tag="pe", name="pep")
                for j in range(2):
                    nc.tensor.matmul(
                        p[:, j * 512:(j + 1) * 512],
                        qkT[4 + h // 2][pb0:pb0 + 64, kt * 128:(kt + 1) * 128],
                        qkT[h // 2][pb0:pb0 + 64,
                                    qh * 1024 + j * 512: qh * 1024 + (j + 1) * 512],
                        start=True, stop=True,
                    )
                at = attp.tile([128, 1024], BF16, tag="att", name="at")
                nc.scalar.activation(at[:], p[:], EXPF)
                if prev_at is not None:
                    for j, o in enumerate((o0, o1)):
                        nc.tensor.matmul(
                            o[0:65, :],
                            v_t[kt - 1][:, h, :],
                            prev_at[:, j * 512:(j + 1) * 512],
                            start=(kt - 1 == 0), stop=False,
                        )
                prev_at = at
                next(filler, None)
                if kt % 4 == 0:
                    next(dve_filler, None)
            for j, o in enumerate((o0, o1)):
                nc.tensor.matmul(
                    o[0:65, :],
                    v_t[NT_N - 1][:, h, :],
                    prev_at[:, j * 512:(j + 1) * 512],
                    start=False, stop=True,
                )
            for j, o in enumerate((o0, o1)):
                qg = qh * 2 + j
                r = h * 4 + qg
                # denominator row -> DRAM bounce
                st = stage.tile([128, 512], F32, tag="st", name="st")
                nc.vector.tensor_copy(st[64:65, :], o[64:65, :])
                nc.gpsimd.dma_start(den_dram[r:r + 1, :], st[64:65, :])
                # raw out^T -> sbuf bf16 (packed 2 heads / tile)
                nc.vector.tensor_copy(
                    outT_raw[h // 2][pb0:pb0 + 64, qg * 512:(qg + 1) * 512],
                    o[0:64, :],
                )
            if h == 7:
                # tail head: den reload + reciprocal + normalize per q-half so
                # proj can start on the lower q range while the upper half of
                # attention drains.  (reload/recip are idempotent recomputes.)
                den_recip(30 if qh == 0 else 32)
                for qg in (2 * qh, 2 * qh + 1):
                    norm_step(7, qg)
        if h % 2 == 1 and h < 7:
            # pair p complete: reload den rows 0..(p+1)*8 (old rows identical)
            # and recompute the full reciprocal tile -- idempotent, keeps all
            # partition starts 32-aligned.
            p_ = h // 2
            den_recip((p_ + 1) * 8)
            for _ in dve_filler:  # drain previous normalize batch
                pass
            dve_filler = normalize_gen((2 * p_, 2 * p_ + 1))
        elif h == 6:
            den_recip(28)
            for _ in dve_filler:
                pass
            dve_filler = normalize_gen((6,))
      for _ in filler:
        pass
      for _ in dve_filler:
        pass

      # ---- phase 3: partial proj  y = outT_n^T @ wp ----
      for nt in range(NT_N):
        ys = yp.tile([128, 1024], F32, tag="y", name="ys")
        for ng in range(2):
            p = po.tile([128, 512], F32, tag="po", name="pp")
            for t4 in range(4):
                nc.tensor.matmul(
                    p[:],
                    outT_n[t4][:, nt * 128:(nt + 1) * 128],
                    wp_t[t4][:, ng * 512:(ng + 1) * 512],
                    start=(t4 == 0), stop=(t4 == 3),
                )
            if ng == 0:
                nc.vector.tensor_copy(ys[:, 0:512], p[:])
            else:
                nc.scalar.copy(ys[:, 512:1024], p[:])
        nc.sync.dma_start(y[nt * 128:(nt + 1) * 128, :], ys[:])


_CACHE = {}


def _get_nc(repeat=1):
    key = f"nc{repeat}"
    if key not in _CACHE:
        nc = bacc.Bacc("TRN2", target_bir_lowering=False, debug=False, num_devices=8)
        xT = nc.dram_tensor("xT", [KAUG, N], BF16, kind="ExternalInput")
        wqk = nc.dram_tensor("wqk", [KAUG, 1024], BF16, kind="ExternalInput")
        wv = nc.dram_tensor("wv", [KAUG, 512], BF16, kind="ExternalInput")
        wp = nc.dram_tensor("wp", [512, 1024], BF16, kind="ExternalInput")
        y = nc.dram_tensor("y", [N, EMB], F32, kind="ExternalOutput")
        with tile.TileContext(nc) as tc:
            from contextlib import ExitStack
            with ExitStack() as es:
                _build_kernel(es, tc, nc, xT.ap(), wqk.ap(), wv.ap(), wp.ap(),
                              y.ap(), repeat=repeat)
        nc.compile()
        _CACHE[key] = nc
    return _CACHE[key]


def make_in_maps(x, w_qkv, b_qkv, w_proj):
    """Host-side shard prep: per-core bf16 operands with folded biases/scale."""
    x = np.asarray(x, np.float32)
    w_qkv = np.asarray(w_qkv, np.float32)
    b_qkv = np.asarray(b_qkv, np.float32)
    w_proj = np.asarray(w_proj, np.float32)
    scale = 1.0 / np.sqrt(HD)

    in_maps = []
    for c in range(8):
        b, g = divmod(c, 2)
        heads = range(g * HPC, (g + 1) * HPC)

        xT_aug = np.zeros((KAUG, N), np.float32)
        xT_aug[0:EMB, :] = x[b].T
        xT_aug[EMB, :] = 1.0

        wqk = np.zeros((KAUG, 1024), np.float32)
        wv = np.zeros((KAUG, 512), np.float32)
        for hl, H in enumerate(heads):
            qs, ks, vs = H * HD, EMB + H * HD, 2 * EMB + H * HD
            # q cols: m-tile hl//2, partition half hl%2; k cols: m-tile 4+hl//2
            qc = (hl // 2) * 128 + (hl % 2) * 64
            kc = 512 + qc
            wqk[0:EMB, qc:qc + 64] = w_qkv[qs:qs + HD, :].T * scale
            wqk[EMB, qc:qc + 64] = b_qkv[qs:qs + HD] * scale
            wqk[0:EMB, kc:kc + 64] = w_qkv[ks:ks + HD, :].T
            wqk[EMB, kc:kc + 64] = b_qkv[ks:ks + HD]
            wv[0:EMB, hl * 64:(hl + 1) * 64] = w_qkv[vs:vs + HD, :].T
            wv[EMB, hl * 64:(hl + 1) * 64] = b_qkv[vs:vs + HD]

        wp = w_proj[:, g * 512:(g + 1) * 512].T.copy()

        in_maps.append({
            "xT": xT_aug.astype(bf16),
            "wqk": wqk.astype(bf16),
            "wv": wv.astype(bf16),
            "wp": wp.astype(bf16),
        })
    return in_maps


def kernel(x, w_qkv, b_qkv, w_proj, b_proj):
    x = np.asarray(x, np.float32)
    b_proj = np.asarray(b_proj, np.float32)
    nc = _get_nc()
    in_maps = make_in_maps(x, w_qkv, b_qkv, w_proj)
    res = run_bass_kernel_spmd(nc, in_maps, core_ids=list(range(8)))
    out = np.empty((B, N, EMB), np.float32)
    for b in range(B):
        out[b] = res.results[2 * b]["y"] + res.results[2 * b + 1]["y"] + b_proj
    return out

